# revision 23
# baseline (speedup 1.0000x reference)
"""Trainium2 Bass kernel: DifferentiableAddressingHead (NTM-style addressing).

Sharding: pure data parallelism over the batch axis. Each of the 8
NeuronCores processes 64 of the 512 batch rows; the tiny dense weights are
replicated. No collectives needed.

Per-core dataflow (BL=64 local batches, M=4096 memory slots, D=64):
  Stage A (small): controller projections on PE (query + beta/gate/shift/
    gamma heads), activations on ACT, query-norm folded into a per-batch
    "bscale" so the raw (unnormalized) query is used against memory.
  Stage B (bulk, per batch b): memory[b] loaded as [128, 2048] tiles
    (m-rows on partitions, 32 m-groups x 64 d on the free dim).
      - ACT: elementwise Square (for row norms)
      - PE:  per-b broadcast of q into PSUM  (ones[1,128]^T @ q[1,64])
      - DVE: product = mem * q (stride-0 broadcast view), then two grouped
             tensor_reduce(axis=X) ops: dot[128,32], normsq[128,32]
      - DMA: reorganize dot/normsq into row b of [64, 4096] staging tiles
  Stage C (batched, [64, 4096], batch on partitions): cosine sim, softmax
    (no max-subtraction: |beta*sim| <= beta, bounded), gate mix with
    previous weights, 3-tap circular conv, sharpening via exp(gamma*ln),
    final normalization. Output DMA'd back contiguous.
"""

from contextlib import ExitStack

import numpy as np

import concourse.bass as bass
import concourse.tile as tile
from concourse import masks, mybir

B, M, D, C = 512, 4096, 64, 256
NCORES = 8
BL = B // NCORES  # 64 batch rows per core
NSHIFT = 3
EPS = 1e-8

F32 = mybir.dt.float32
AF = mybir.ActivationFunctionType
ALU = mybir.AluOpType
AX = mybir.AxisListType

P = 128          # SBUF partitions
G = M // P       # 32 m-rows per partition
FD = G * D       # 2048 free elements per memory tile


def _body(tc, nc, mem, cs, prev, Wk, bb, wheads, out):
    ctx = tc._body_ctx

    const = ctx.enter_context(tc.tile_pool(name="const", bufs=1))
    small = ctx.enter_context(tc.tile_pool(name="small", bufs=1))
    spsum = ctx.enter_context(tc.tile_pool(name="spsum", bufs=1, space="PSUM"))
    mem_pool = ctx.enter_context(tc.tile_pool(name="mem", bufs=3))
    work = ctx.enter_context(tc.tile_pool(name="work", bufs=2))
    red = ctx.enter_context(tc.tile_pool(name="red", bufs=4))
    qrep_pool = ctx.enter_context(tc.tile_pool(name="qrep", bufs=3, space="PSUM"))
    big = ctx.enter_context(tc.tile_pool(name="big", bufs=1))

    # ---------------- constants ----------------
    # NOTE: every PE matmul operand is staged through a DVE copy so the
    # lowered LDWEIGHTS carries at most ONE semaphore wait (walrus
    # S3_LW_STRUCT limit: "Too many sync wait commands" otherwise).
    ident0 = const.tile([BL, BL], F32, tag="ident0")
    masks.make_identity(nc, ident0[:])
    ident = const.tile([BL, BL], F32, tag="ident")
    nc.vector.tensor_copy(ident[:], ident0[:])
    ones0 = const.tile([1, P], F32, tag="ones0")
    nc.gpsimd.memset(ones0[:], 1.0)
    ones_row = const.tile([1, P], F32, tag="ones")
    nc.vector.tensor_copy(ones_row[:], ones0[:])

    # ---------------- load controller + weights ----------------
    # controller + host-prepacked weights (one DMA per staged tile so the
    # DVE staging copy waits on a single DMA-queue semaphore)
    cs_raw = small.tile([BL, C], F32, tag="csraw")
    nc.sync.dma_start(cs_raw[:], cs[:])
    cs_sb = small.tile([BL, C], F32, tag="cs")
    nc.vector.tensor_copy(cs_sb[:], cs_raw[:])

    wk_raw = small.tile([P, 2 * D], F32, tag="wkraw")
    nc.sync.dma_start(wk_raw[:], Wk[:])          # wk_pack [128, 128]
    wk_sb = small.tile([P, 2 * D], F32, tag="wk")
    nc.vector.tensor_copy(wk_sb[:], wk_raw[:])

    wh_raw = small.tile([P, 12], F32, tag="whraw")
    nc.sync.dma_start(wh_raw[:], wheads[:])      # wh_pack [128, 12]
    wh_sb = small.tile([P, 12], F32, tag="wh")
    nc.vector.tensor_copy(wh_sb[:], wh_raw[:])

    brow_raw = small.tile([1, 6], F32, tag="browraw")
    nc.sync.dma_start(brow_raw[:], bb[:])        # b_pack [1, 6]
    brow = small.tile([1, 6], F32, tag="brow")
    nc.vector.tensor_copy(brow[:], brow_raw[:])

    # ---------------- transpose cs -> csT [C(2x128 part), BL] ----------------
    csT = small.tile([P, 2 * BL], F32, tag="csT")
    for ci in range(2):
        t_ps = spsum.tile([P, BL], F32, tag="tps")
        nc.tensor.transpose(t_ps[:], cs_sb[:, ci * P:(ci + 1) * P], ident[:])
        nc.vector.tensor_copy(csT[:, ci * BL:(ci + 1) * BL], t_ps[:])

    # ---------------- query + heads on PE ----------------
    q_ps = spsum.tile([BL, D], F32, tag="qps")
    nc.tensor.matmul(q_ps[:], csT[:, 0:BL], wk_sb[:, 0:D], start=True, stop=False)
    nc.tensor.matmul(q_ps[:], csT[:, BL:2 * BL], wk_sb[:, D:2 * D],
                     start=False, stop=True)
    q_sb = small.tile([BL, D], F32, tag="qsb")
    nc.vector.tensor_copy(q_sb[:], q_ps[:])

    h_ps = spsum.tile([BL, 6], F32, tag="hps")
    nc.tensor.matmul(h_ps[:], csT[:, 0:BL], wh_sb[:, 0:6], start=True, stop=False)
    nc.tensor.matmul(h_ps[:], csT[:, BL:2 * BL], wh_sb[:, 6:12],
                     start=False, stop=False)
    # += bias (broadcast row via ones)
    nc.tensor.matmul(h_ps[:], ones_row[0:1, 0:BL], brow[:], start=False, stop=True)
    h_sb = small.tile([BL, 6], F32, tag="hsb")
    nc.vector.tensor_copy(h_sb[:], h_ps[:])

    # ---------------- per-batch scalars ----------------
    qsq = small.tile([BL, D], F32, tag="qsq")
    qn2 = small.tile([BL, 1], F32, tag="qn2")
    nc.scalar.activation(qsq[:], q_sb[:], AF.Square, accum_out=qn2[:])
    qnorm = small.tile([BL, 1], F32, tag="qnorm")
    nc.scalar.activation(qnorm[:], qn2[:], AF.Sqrt)
    qne = small.tile([BL, 1], F32, tag="qne")
    nc.vector.tensor_scalar(qne[:], qnorm[:], EPS, None, op0=ALU.add)
    qrecip = small.tile([BL, 1], F32, tag="qrecip")
    nc.vector.reciprocal(qrecip[:], qne[:])

    # bscale = (softplus(h0)+1) / (|q|+eps); softplus = ln(1+exp(x)),
    # safe without clamping since |h| <= ~8 here
    spe = small.tile([BL, 1], F32, tag="spe")
    nc.scalar.activation(spe[:], h_sb[:, 0:1], AF.Exp)
    spb = small.tile([BL, 1], F32, tag="spb")
    nc.scalar.activation(spb[:], spe[:], AF.Ln, bias=1.0)
    bscale = small.tile([BL, 1], F32, tag="bscale")
    nc.vector.tensor_scalar(bscale[:], spb[:], 1.0, qrecip[:],
                            op0=ALU.add, op1=ALU.mult)

    g_t = small.tile([BL, 1], F32, tag="gate")
    nc.scalar.activation(g_t[:], h_sb[:, 1:2], AF.Sigmoid)
    omg = small.tile([BL, 1], F32, tag="omg")
    nc.scalar.activation(omg[:], g_t[:], AF.Copy, bias=1.0, scale=-1.0)

    e3 = small.tile([BL, NSHIFT], F32, tag="e3")
    nc.scalar.activation(e3[:], h_sb[:, 2:5], AF.Exp)
    ssum = small.tile([BL, 1], F32, tag="ssum")
    nc.vector.tensor_reduce(ssum[:], e3[:], axis=AX.X, op=ALU.add)
    srec = small.tile([BL, 1], F32, tag="srec")
    nc.vector.reciprocal(srec[:], ssum[:])
    sk = small.tile([BL, NSHIFT], F32, tag="sk")
    nc.vector.tensor_scalar(sk[:], e3[:], srec[:], None, op0=ALU.mult)

    gse = small.tile([BL, 1], F32, tag="gse")
    nc.scalar.activation(gse[:], h_sb[:, 5:6], AF.Exp)
    gsp = small.tile([BL, 1], F32, tag="gsp")
    nc.scalar.activation(gsp[:], gse[:], AF.Ln, bias=1.0)
    gamma = small.tile([BL, 1], F32, tag="gamma")
    nc.vector.tensor_scalar(gamma[:], gsp[:], 1.0, None, op0=ALU.add)

    # ---------------- broadcast all queries to all partitions ----------------
    # q_sb [64b, 64d] -> qflat [1, 4096] (partition-major gather via DMA),
    # then ones[1,128]^T @ qflat chunks -> qrep_all [128, 4096] in SBUF.
    qflat_raw = small.tile([1, BL * D], F32, tag="qflatraw")
    nc.sync.dma_start(qflat_raw[:], q_sb[:])
    qflat = small.tile([1, BL * D], F32, tag="qflat")
    nc.vector.tensor_copy(qflat[:], qflat_raw[:])
    qrep_all = small.tile([P, BL * D], F32, tag="qrepall")
    for ci in range(BL * D // 512):
        qc_ps = qrep_pool.tile([P, 512], F32, tag="qcps")
        nc.tensor.matmul(qc_ps[:], ones_row[:], qflat[:, ci * 512:(ci + 1) * 512],
                         start=True, stop=True)
        nc.vector.tensor_copy(qrep_all[:, ci * 512:(ci + 1) * 512], qc_ps[:])

    # ---------------- stage B: bulk similarity ----------------
    d_dot = big.tile([BL, M], F32, tag="ddot")
    d_nsq = big.tile([BL, M], F32, tag="dnsq")

    for b in range(BL):
        mem_t = mem_pool.tile([P, FD], F32, tag="memt")
        nc.sync.dma_start(mem_t[:], mem[b].rearrange("(p g) d -> p (g d)", p=P))

        mem3 = mem_t[:].rearrange("p (g d) -> p g d", d=D)
        qv = qrep_all[:, b * D:(b + 1) * D].unsqueeze(1).broadcast_to([P, G, D])

        prod = work.tile([P, FD], F32, tag="prod")
        nc.vector.tensor_tensor(prod[:].rearrange("p (g d) -> p g d", d=D),
                                mem3, qv, op=ALU.mult)

        sq = work.tile([P, FD], F32, tag="sq")
        nc.scalar.activation(sq[:], mem_t[:], AF.Square)

        dot_b = red.tile([P, G], F32, tag="dotb")
        nc.vector.tensor_reduce(dot_b[:], prod[:].rearrange("p (g d) -> p g d", d=D),
                                axis=AX.X, op=ALU.add)
        nsq_b = red.tile([P, G], F32, tag="nsqb")
        nc.vector.tensor_reduce(nsq_b[:], sq[:].rearrange("p (g d) -> p g d", d=D),
                                axis=AX.X, op=ALU.add)

        # reorganize into row b (m = 32*p + g ordering matches partition-major)
        nc.sync.dma_start(d_dot[b:b + 1, :], dot_b[:])
        nc.sync.dma_start(d_nsq[b:b + 1, :], nsq_b[:])

    # ---------------- stage C: batched postprocessing [BL, M] ----------------
    prev_t = big.tile([BL, M], F32, tag="prevt")
    nc.sync.dma_start(prev_t[:], prev[:])

    # sim = dot * rsqrt(nsq) * bscale   (in place in d_dot)
    nc.scalar.activation(d_nsq[:], d_nsq[:], AF.Sqrt)
    nc.vector.reciprocal(d_nsq[:], d_nsq[:])
    nc.vector.tensor_tensor(d_dot[:], d_dot[:], d_nsq[:], op=ALU.mult)
    nc.vector.tensor_scalar(d_dot[:], d_dot[:], bscale[:], None, op0=ALU.mult)

    # softmax numerator (bounded logits; no max subtraction needed)
    nc.scalar.activation(d_dot[:], d_dot[:], AF.Exp)
    esum = small.tile([BL, 1], F32, tag="esum")
    nc.vector.tensor_reduce(esum[:], d_dot[:], axis=AX.X, op=ALU.add)
    erec = small.tile([BL, 1], F32, tag="erec")
    nc.vector.reciprocal(erec[:], esum[:])
    galpha = small.tile([BL, 1], F32, tag="galpha")
    nc.vector.tensor_tensor(galpha[:], g_t[:], erec[:], op=ALU.mult)

    # gated = gate*softmax + (1-gate)*prev   (in place in d_dot)
    nc.vector.tensor_scalar(d_dot[:], d_dot[:], galpha[:], None, op0=ALU.mult)
    nc.vector.scalar_tensor_tensor(d_dot[:], prev_t[:], omg[:], d_dot[:],
                                   op0=ALU.mult, op1=ALU.add)

    # 3-tap circular conv into conv buffer (reuses d_nsq slot via tag)
    conv = big.tile([BL, M], F32, tag="dnsq")
    sk0, sk1, sk2 = sk[:, 0:1], sk[:, 1:2], sk[:, 2:3]
    nc.vector.tensor_scalar(conv[:], d_dot[:], sk1, None, op0=ALU.mult)
    nc.vector.scalar_tensor_tensor(conv[:, 1:M], d_dot[:, 0:M - 1], sk0,
                                   conv[:, 1:M], op0=ALU.mult, op1=ALU.add)
    nc.vector.scalar_tensor_tensor(conv[:, 0:1], d_dot[:, M - 1:M], sk0,
                                   conv[:, 0:1], op0=ALU.mult, op1=ALU.add)
    nc.vector.scalar_tensor_tensor(conv[:, 0:M - 1], d_dot[:, 1:M], sk2,
                                   conv[:, 0:M - 1], op0=ALU.mult, op1=ALU.add)
    nc.vector.scalar_tensor_tensor(conv[:, M - 1:M], d_dot[:, 0:1], sk2,
                                   conv[:, M - 1:M], op0=ALU.mult, op1=ALU.add)

    # sharpen: (conv+eps)^gamma = exp(gamma*ln(conv+eps)), then normalize
    nc.scalar.activation(conv[:], conv[:], AF.Ln, bias=EPS)
    nc.scalar.activation(conv[:], conv[:], AF.Exp, scale=gamma[:])
    psm = small.tile([BL, 1], F32, tag="psm")
    nc.vector.tensor_reduce(psm[:], conv[:], axis=AX.X, op=ALU.add)
    psme = small.tile([BL, 1], F32, tag="psme")
    nc.vector.tensor_scalar(psme[:], psm[:], EPS, None, op0=ALU.add)
    prc = small.tile([BL, 1], F32, tag="prc")
    nc.vector.reciprocal(prc[:], psme[:])
    nc.vector.tensor_scalar(conv[:], conv[:], prc[:], None, op0=ALU.mult)

    nc.sync.dma_start(out[:], conv[:])


def build(split_waits=True):
    nc = bass.Bass()
    mem = nc.dram_tensor("memory", [BL, M, D], F32, kind="ExternalInput")
    cs = nc.dram_tensor("controller_state", [BL, C], F32, kind="ExternalInput")
    prev = nc.dram_tensor("previous_weights", [BL, M], F32, kind="ExternalInput")
    # host-prepacked weights (see _make_in_maps)
    wk_pack = nc.dram_tensor("wk_pack", [P, 2 * D], F32, kind="ExternalInput")
    wh_pack = nc.dram_tensor("wh_pack", [P, 12], F32, kind="ExternalInput")
    b_pack = nc.dram_tensor("b_pack", [1, 6], F32, kind="ExternalInput")
    out = nc.dram_tensor("out", [BL, M], F32, kind="ExternalOutput")

    # register EPS so float biases on ACT instructions resolve to a const AP
    eps_t = nc.alloc_sbuf_tensor("const-f32-eps", [128, 1], F32)
    nc.gpsimd.memset(eps_t.ap(), EPS)
    nc.const_aps.aps[(F32, EPS)] = eps_t.ap()
    nc.all_engine_barrier()

    with tile.TileContext(nc) as tc:
        with ExitStack() as ctx:
            tc._body_ctx = ctx
            _body(tc, nc, mem, cs, prev, wk_pack, b_pack, wh_pack, out)
    if split_waits:
        _split_multiwait(nc)
    return nc


def _split_multiwait(nc, max_waits=1):
    """Walrus ISA structs encode a limited number of semaphore waits per
    instruction ("Too many sync wait commands"). Move all but one wait of
    any multi-wait instruction onto same-engine InstNoOp instructions
    inserted directly before it."""
    for fn in nc.m.functions:
        for blk in fn.blocks:
            insts = blk.instructions
            idx = 0
            while idx < len(insts):
                inst = insts[idx]
                si = inst.sync_info
                if si is not None and len(si.on_wait) > max_waits:
                    waits = list(si.on_wait)
                    extra, keep = waits[:-max_waits], waits[-max_waits:]
                    for w in extra:
                        nop = mybir.InstNoOp(
                            name=nc.get_next_instruction_name(),
                            sync_info=mybir.SyncInfo(on_wait=[w], on_update=[]),
                            bass_nofuse=True,
                            engine=inst.engine,
                        )
                        insts.insert(idx, nop)
                        idx += 1
                    inst.sync_info = mybir.SyncInfo(
                        on_wait=keep, on_update=list(si.on_update))
                idx += 1


_NC = None


def _get_nc():
    global _NC
    if _NC is None:
        _NC = build()
    return _NC


def _make_in_maps(inputs):
    full = {k: np.ascontiguousarray(np.asarray(v, dtype=np.float32))
            for k, v in inputs.items()}
    # host-side repack of the tiny replicated weights into SBUF tile layouts
    wk_pack = np.ascontiguousarray(
        np.concatenate([full["Wk"][0:P, :], full["Wk"][P:C, :]], axis=1))
    wh = np.concatenate(
        [full["Wb"], full["Wgate"], full["Ws"], full["Wg"]], axis=1)  # [C, 6]
    wh_pack = np.ascontiguousarray(np.concatenate([wh[0:P], wh[P:C]], axis=1))
    b_pack = np.ascontiguousarray(np.concatenate(
        [full["bb"].reshape(-1), full["bgate"].reshape(-1),
         full["bs"].reshape(-1), full["bg"].reshape(-1)]).reshape(1, 6))
    in_maps = []
    for c in range(NCORES):
        sl = slice(c * BL, (c + 1) * BL)
        in_maps.append({
            "memory": full["memory"][sl],
            "controller_state": full["controller_state"][sl],
            "previous_weights": full["previous_weights"][sl],
            "wk_pack": wk_pack, "wh_pack": wh_pack, "b_pack": b_pack,
        })
    return in_maps


def run(inputs, **kwargs):
    from concourse.bass_utils import run_bass_kernel_spmd
    nc = _get_nc()
    res = run_bass_kernel_spmd(nc, _make_in_maps(inputs),
                               list(range(NCORES)), **kwargs)
    out = np.concatenate([res.results[c]["out"] for c in range(NCORES)], axis=0)
    return out.astype(np.float32), res


def kernel(**inputs):
    out, _ = run(inputs)
    return out
